# revision 4
# baseline (speedup 1.0000x reference)
"""AutoCorrelation (Autoformer-style) Bass kernel for one TRN2 chip (8 NeuronCores).

Math: the reference computes, per (b, h):
    corr = irfft(rfft(q, axis=-1) * conj(rfft(k, axis=-1)), n=L)   # [L, L]
    weights = softmax(corr - mean_h(corr), axis=-1)
    Vt = v @ weights                                                # [d, L]
Since the rfft is over the d=64 channel axis and the irfft zero-pads 33 bins
to L=2048, corr[s, :] is a rank-<=66 function of t; the DC term is constant
over t and cancels in softmax.  With products (re*re, im*im, im*re, re*im)
collapsed to 64 cos/sin coefficient rows, the logits are an exact K=64 matmul
against a fixed cos/sin basis — no [L, L] tensor ever exists in DRAM.
Sharding: head h -> core h (both batches); only the head-mean of the 64 x 2048
coefficient matrix needs an AllReduce (0.25 MB bf16 per batch).  A tiny dummy
AllReduce issued at kernel start absorbs the collective firmware's first-call
latency while the coefficient phase runs.

K=64 logits matmuls are row-packed (two concurrent 64-row tiles in the PE
array via base_partition 0/64) and the delay-aggregation matmuls are
column-packed (Vt stored as [128, 1024]: partitions 0-63 hold t 0:1024,
partitions 64-127 hold t 1024:2048), so the PE runs both members of each pair
concurrently.  The softmax exp is split between ScalarE (table exp) and
VectorE (custom DVE op: exp(x) ~= (c0 + x(c1 + x c2))^8, valid since logits
are bounded ~|1.5|), both with fused free-dim accumulation for the softmax
denominator.
"""

import sys
from operator import add as _op_add

sys.path.insert(0, "/opt/trn_rl_repo")

import numpy as np
import ml_dtypes

from concourse import bass, bacc, mybir, tile
from concourse import dve_ops
from concourse.dve_spec import Spec, Src0, C0, C1, C2, Zero, sq, lower
from concourse.dve_uop import DveOpSpec
from concourse.bass_utils import run_bass_kernel_spmd

B, L, E, H, D = 2, 2048, 512, 8, 64
NF = 32          # frequencies 1..32 of the 64-point rfft (DC dropped)
NCOMP = 4 * NF   # 128 raw product rows
NCC = 2 * NF     # 64 compressed coefficient rows (cos, sin)
NCORES = 8
SC = L // 128    # 16 s-chunks of 128 rows
BF16 = mybir.dt.bfloat16
F32 = mybir.dt.float32

# minimax quadratic p(z) for e^z on z = x/8, |x| <= 1.68; exp(x) ~= p(x)^8
EXP_C = (0.99970171, 0.12580122, 0.00795605)

TRACE = False
LAST_RESULT = None

_COMPILED = None
_EXP_OP = None


def _register_exp_op():
    global _EXP_OP
    if _EXP_OP is not None:
        return _EXP_OP
    for o in dve_ops.OPS:
        if o.name == "EXP8_ANT":
            _EXP_OP = o
            return o

    body = sq(sq(sq(C0 + Src0 * (C1 + Src0 * C2))))

    def _ref(in0, in1, c0, c1, c2):
        x = in0.astype(np.float32)
        b = (((c0 + x * (c1 + x * c2)) ** 8)).astype(np.float32)
        return b, b.reshape(b.shape[0], -1).sum(axis=-1, keepdims=True)

    spec = Spec(body=body, accum=_op_add, accum_init=Zero, reference=_ref)
    opcode = dve_ops._CUSTOM_DVE_ROW_BASE + len(dve_ops.OPS)
    dve_ops._SUB_OPCODE_FOR_NAME["EXP8_ANT"] = opcode
    shas = {}
    for ver in ("v3", "v4"):
        shas[ver] = DveOpSpec(
            name="EXP8_ANT", opcode=opcode, uops=lower(spec, ver=ver), rd1_en=False
        ).sha(ver)
    op = dve_ops.DveOp("EXP8_ANT", spec, subdim=False, uops_sha=shas)
    dve_ops.OPS.append(op)
    dve_ops.CUSTOM_DVE_SPECS[op.name] = spec
    _EXP_OP = op
    return op


def _constants():
    c = np.arange(D)
    f = np.arange(1, NF + 1)
    ang = 2 * np.pi * np.outer(c, f) / D
    fcos = np.cos(ang)       # Re X_f   = sum_c q_c cos
    fsin = -np.sin(ang)      # Im X_f   = -sum_c q_c sin
    w = 2.0 / L              # irfft weight for interior bins
    fx = np.concatenate([fcos * w, fsin * w, fsin * w, fcos * w], axis=1)  # [64, 128]
    fy = np.concatenate([fcos, fsin, fcos, fsin], axis=1)                  # [64, 128]
    t = np.arange(L)
    angt = 2 * np.pi * np.outer(f, t) / L
    cosb, sinb = np.cos(angt), np.sin(angt)
    basis64 = np.concatenate([cosb, -sinb], axis=0)                        # [64, 2048]
    basisdup = np.concatenate([basis64, basis64], axis=0)                  # [128, 2048]
    # compression: Ccs[0:32] = P[0:32] + P[32:64]  (re*re + im*im -> cos)
    #              Ccs[32:64] = P[64:96] - P[96:128] (im*re - re*im -> -sin)
    mcomp = np.zeros((NCOMP, NCC), np.float32)
    for m in range(32):
        mcomp[m, m] = 1.0
        mcomp[m + 32, m] = 1.0
        mcomp[m + 64, m + 32] = 1.0
        mcomp[m + 96, m + 32] = -1.0
    bf = ml_dtypes.bfloat16
    return fx.astype(bf), fy.astype(bf), basisdup.astype(bf), mcomp.astype(bf)


def _build():
    exp_op = _register_exp_op()
    nc = bacc.Bacc("TRN2", target_bir_lowering=False, debug=False, num_devices=NCORES)

    qT_d = nc.dram_tensor("qT", [B, D, L], BF16, kind="ExternalInput")
    kT_d = nc.dram_tensor("kT", [B, D, L], BF16, kind="ExternalInput")
    v_d = nc.dram_tensor("v", [B, L, D], F32, kind="ExternalInput")
    fx_d = nc.dram_tensor("fx", [D, NCOMP], BF16, kind="ExternalInput")
    fy_d = nc.dram_tensor("fy", [D, NCOMP], BF16, kind="ExternalInput")
    basis_d = nc.dram_tensor("basis2", [NCOMP, L], BF16, kind="ExternalInput")
    mcomp_d = nc.dram_tensor("mcomp", [NCOMP, NCC], BF16, kind="ExternalInput")
    out_d = nc.dram_tensor("out", [B, D, L], F32, kind="ExternalOutput")

    rg = [list(range(NCORES))]

    with tile.TileContext(nc) as tc:
        with (
            tc.tile_pool(name="consts", bufs=1) as consts,
            tc.tile_pool(name="qk", bufs=2) as qk_pool,
            tc.tile_pool(name="vv", bufs=2) as v_pool,
            tc.tile_pool(name="xy", bufs=2) as xy_pool,
            tc.tile_pool(name="cf", bufs=2) as cf_pool,
            tc.tile_pool(name="cs", bufs=2) as cs_pool,
            tc.tile_pool(name="cd", bufs=2) as cd_pool,
            tc.tile_pool(name="wts", bufs=6) as w_pool,
            tc.tile_pool(name="small", bufs=12) as s_pool,
            tc.tile_pool(name="outp", bufs=2) as out_pool,
            tc.tile_pool(name="ps_log", bufs=3, space="PSUM") as ps_log,
            tc.tile_pool(name="ps_vt", bufs=1, space="PSUM") as ps_vt,
            tc.tile_pool(name="dram", bufs=1, space="DRAM") as dram,
        ):
            # Warm up the collective firmware immediately; nothing reads the
            # result — by the time the real AllReduces fire, ncfw is awake.
            warm_in = dram.tile([128, 8], BF16, name="warm_in")
            warm_out = dram.tile([128, 8], BF16, addr_space="Shared", name="warm_out")
            nc.gpsimd.collective_compute(
                "AllReduce", mybir.AluOpType.add, replica_groups=rg,
                ins=[warm_in[:].opt()], outs=[warm_out[:].opt()],
            )

            fx_sb = consts.tile([D, NCOMP], BF16)
            fy_sb = consts.tile([D, NCOMP], BF16)
            basis_sb = consts.tile([NCOMP, L], BF16)
            mcomp_sb = consts.tile([NCOMP, NCC], BF16)
            nc.sync.dma_start(out=fx_sb[:], in_=fx_d[:])
            nc.sync.dma_start(out=fy_sb[:], in_=fy_d[:])
            nc.sync.dma_start(out=basis_sb[:], in_=basis_d[:])
            nc.sync.dma_start(out=mcomp_sb[:], in_=mcomp_d[:])

            cc_in = [dram.tile([NCC, L], BF16, name=f"cc_in{b}") for b in range(B)]
            cc_out = [
                dram.tile([NCC, L], BF16, addr_space="Shared", name=f"cc_out{b}")
                for b in range(B)
            ]

            # ---- Phase 1: per-b compressed coefficients Ccs [64, L] + AllReduce ----
            ccs_local = []
            for b in range(B):
                qT_sb = qk_pool.tile([D, L], BF16, tag="qT")
                kT_sb = qk_pool.tile([D, L], BF16, tag="kT")
                nc.sync.dma_start(out=qT_sb[:], in_=qT_d[b])
                nc.sync.dma_start(out=kT_sb[:], in_=kT_d[b])

                xt2 = xy_pool.tile([NCOMP, L], BF16, tag="xt2")
                yt2 = xy_pool.tile([NCOMP, L], BF16, tag="yt2")
                for src_sb, fmat, dst, cast_eng in (
                    (qT_sb, fx_sb, xt2, "scalar"),
                    (kT_sb, fy_sb, yt2, "vector"),
                ):
                    for j in range(2):  # s-halves of 1024
                        ps = ps_log.tile([NCOMP, 1024], F32, tag="log")
                        for q in range(2):
                            nc.tensor.matmul(
                                ps[:, q * 512:(q + 1) * 512],
                                fmat[:],
                                src_sb[:, j * 1024 + q * 512: j * 1024 + (q + 1) * 512],
                                start=True, stop=True,
                            )
                        if cast_eng == "scalar":
                            nc.scalar.copy(dst[:, j * 1024:(j + 1) * 1024], ps[:])
                        else:
                            nc.vector.tensor_copy(dst[:, j * 1024:(j + 1) * 1024], ps[:])

                cf = cf_pool.tile([NCOMP, L], BF16, tag="cfull")
                nc.vector.tensor_mul(cf[:], xt2[:], yt2[:])

                ccs = cs_pool.tile([NCC, L], BF16, tag="ccs")
                for j in range(2):
                    ps = ps_log.tile([NCC, 1024], F32, tag="log")
                    for q in range(2):
                        nc.tensor.matmul(
                            ps[:, q * 512:(q + 1) * 512],
                            mcomp_sb[:],
                            cf[:, j * 1024 + q * 512: j * 1024 + (q + 1) * 512],
                            start=True, stop=True,
                        )
                    nc.scalar.copy(ccs[:, j * 1024:(j + 1) * 1024], ps[:NCC, :])
                ccs_local.append(ccs)
                nc.sync.dma_start(out=cc_in[b][:], in_=ccs[:])

                nc.gpsimd.collective_compute(
                    "AllReduce", mybir.AluOpType.add, replica_groups=rg,
                    ins=[cc_in[b][:].opt()], outs=[cc_out[b][:].opt()],
                )

            # cd2: mean-subtracted coefficients duplicated to both partition
            # halves so K=64 logits matmuls can row-pack the PE array.
            cd2 = []
            for b in range(B):
                csum = cs_pool.tile([NCC, L], BF16, tag="csum")
                nc.sync.dma_start(out=csum[:], in_=cc_out[b][:])
                cd64 = cs_pool.tile([NCC, L], BF16, tag="cd64")
                # cd64 = ccs - mean_h = (csum * -1/8) + ccs
                nc.vector.scalar_tensor_tensor(
                    cd64[:], csum[:], -1.0 / NCORES, ccs_local[b][:],
                    op0=mybir.AluOpType.mult, op1=mybir.AluOpType.add,
                )
                cdd = cd_pool.tile([2 * NCC, L], BF16, tag="cd2")
                nc.sync.dma_start(out=cdd[0:NCC, :], in_=cd64[:])
                nc.sync.dma_start(out=cdd[NCC:2 * NCC, :], in_=cd64[:])
                cd2.append(cdd)

            # ---- Phase 2: per-b softmax + delay aggregation ----
            # Vt packed: partitions 0-63 = Vt[:, 0:1024], 64-127 = Vt[:, 1024:2048]
            for b in range(B):
                v_sb = v_pool.tile([128, SC, D], F32, tag="v")
                nc.sync.dma_start(
                    out=v_sb[:], in_=v_d[b].rearrange("(c p) d -> p c d", p=128)
                )
                vt_ps = ps_vt.tile([128, 1024], F32, tag="vt")

                wts_hist = {}
                vts_hist = {}
                sig_hist = {}

                def emit_acc(sc):
                    pwt = wts_hist.pop(sc)
                    pvts = vts_hist.pop(sc)
                    for q in range(2):  # packed pairs: (q, q+2)
                        nc.tensor.matmul(
                            vt_ps[0:D, q * 512:(q + 1) * 512],
                            pvts[:],
                            pwt[0][:, q * 512:(q + 1) * 512],
                            start=(sc == 0), stop=(sc == SC - 1),
                        )
                        nc.tensor.matmul(
                            vt_ps[D:2 * D, q * 512:(q + 1) * 512],
                            pvts[:],
                            pwt[1][:, q * 512:(q + 1) * 512],
                            start=(sc == 0), stop=(sc == SC - 1),
                        )

                def emit_small(sc):
                    sig = sig_hist.pop(sc)
                    sigsum = s_pool.tile([128, 1], F32, tag="sigsum")
                    nc.scalar.add(sigsum[:], sig[:, 0:1], sig[:, 1:2])
                    rcp = s_pool.tile([128, 1], F32, tag="rcp")
                    nc.vector.reciprocal_approx_fast(rcp[:], sigsum[:])
                    vts = s_pool.tile([128, D], BF16, tag="vts")
                    nc.vector.tensor_scalar_mul(vts[:], v_sb[:, sc, :], rcp[:])
                    vts_hist[sc] = vts

                for sc in range(SC):
                    cdt = cd2[b][0:NCC, sc * 128:(sc + 1) * 128]
                    cdb = cd2[b][NCC:2 * NCC, sc * 128:(sc + 1) * 128]
                    lg0 = ps_log.tile([128, 1024], F32, tag="log")
                    lg1 = ps_log.tile([128, 1024], F32, tag="log")
                    for q in range(2):
                        # row-packed pair: h2=0 on PE rows 0-63, h2=1 on 64-127
                        nc.tensor.matmul(
                            lg0[:, q * 512:(q + 1) * 512], cdt,
                            basis_sb[0:NCC, q * 512:(q + 1) * 512],
                            start=True, stop=True,
                        )
                        nc.tensor.matmul(
                            lg1[:, q * 512:(q + 1) * 512], cdb,
                            basis_sb[NCC:2 * NCC, 1024 + q * 512: 1024 + (q + 1) * 512],
                            start=True, stop=True,
                        )
                    if sc >= 2:
                        emit_acc(sc - 2)

                    sig = s_pool.tile([128, 2], F32, tag="sig")
                    wt0 = w_pool.tile([128, 1024], BF16, tag="wt")
                    nc.scalar.activation(
                        wt0[:], lg0[:], mybir.ActivationFunctionType.Exp,
                        accum_out=sig[:, 0:1],
                    )
                    wt1 = w_pool.tile([128, 1024], BF16, tag="wt")
                    nc.vector._custom_dve(
                        exp_op, out=wt1[:], in0=lg1[:],
                        s0=EXP_C[0], s1=EXP_C[1], imm2=EXP_C[2],
                        accum_out=sig[:, 1:2],
                    )
                    wts_hist[sc] = (wt0, wt1)
                    sig_hist[sc] = sig
                    if sc >= 1:
                        emit_small(sc - 1)

                emit_small(SC - 1)
                emit_acc(SC - 2)
                emit_acc(SC - 1)

                out_sb = out_pool.tile([128, 1024], F32, tag="out")
                nc.vector.tensor_copy(out_sb[:], vt_ps[:])
                nc.sync.dma_start(out=out_d[b][:, 0:1024], in_=out_sb[0:D, :])
                nc.sync.dma_start(out=out_d[b][:, 1024:2048], in_=out_sb[D:2 * D, :])

    nc.compile()
    return nc


def _get_compiled():
    global _COMPILED
    if _COMPILED is None:
        _COMPILED = _build()
    return _COMPILED


def kernel(queries, keys, values):
    global LAST_RESULT
    queries = np.asarray(queries, dtype=np.float32)
    keys = np.asarray(keys, dtype=np.float32)
    values = np.asarray(values, dtype=np.float32)

    fx, fy, basisdup, mcomp = _constants()
    bf = ml_dtypes.bfloat16
    warm = np.zeros((128, 8), bf)

    in_maps = []
    for i in range(NCORES):
        sl = slice(i * D, (i + 1) * D)
        in_maps.append({
            "qT": np.ascontiguousarray(queries[:, :, sl].transpose(0, 2, 1)).astype(bf),
            "kT": np.ascontiguousarray(keys[:, :, sl].transpose(0, 2, 1)).astype(bf),
            "v": np.ascontiguousarray(values[:, :, sl]),
            "fx": fx,
            "fy": fy,
            "basis2": basisdup,
            "mcomp": mcomp,
        })

    nc = _get_compiled()
    kw = {"trace_cores": list(range(NCORES))} if TRACE else {}
    res = run_bass_kernel_spmd(nc, in_maps, core_ids=list(range(NCORES)), trace=TRACE, **kw)
    LAST_RESULT = res

    vt_full = np.stack([res.results[i]["out"] for i in range(NCORES)], axis=1)
    # reference: out = transpose(Vt[B,H,d,L], (0,2,1,3)).reshape(B, L, H*d)
    return np.ascontiguousarray(
        vt_full.transpose(0, 2, 1, 3).reshape(B, L, E)
    ).astype(np.float32)
